# revision 1
# baseline (speedup 1.0000x reference)
"""Bidirectional Mamba block on 8 trn2 NeuronCores.

Sharding: d_inner (1536) split 8 ways -> 192 channels/core, held as two
partition chunks (128 + 64). Layout on device is [d partitions, L free]
throughout: it feeds the depthwise conv (per-partition weight scalars), the
x_proj/out_proj matmuls (d = contraction dim = partition dim), and the
selective scan (tensor_tensor_scan runs along the free dim).

Engine split per state n: B_n/C_n rows are DMA-broadcast (step-0 partition
AP from a small f16 DRAM bounce) into [128, L] f16 tiles so the dbu/hc muls
hit the DVE 2x fp16 mode; the scan is DVE-only (1x); the sum over the 16
states runs on the PE as f16 identity-matmuls accumulating into PSUM
(exact fp32).

Cross-core: x_proj contracts over sharded d -> one 640KB AllReduce per
batch. out_proj partials (2,768,1024) are summed on the host.
"""

import contextlib

import numpy as np

import concourse.bass as bass
import concourse.bacc as bacc
import concourse.tile as tile
from concourse import mybir
from concourse.bass_utils import run_bass_kernel_spmd

B, L, DM, DI, DSTATE, DTR, KC = 2, 1024, 768, 1536, 16, 48, 4
NCORES = 8
DCORE = DI // NCORES            # 192
CHS = [(0, 128), (128, 64)]     # (chunk offset in DCORE, partition count)
KT = DM // 128                  # 6 k-tiles for in_proj
F32 = mybir.dt.float32
F32R = mybir.dt.float32r
F16 = mybir.dt.float16
I32 = mybir.dt.int32
AF = mybir.ActivationFunctionType
OP = mybir.AluOpType

PV_CW, PV_CWF, PV_A, PV_AF = 0, 4, 8, 24
PV_CB, PV_CBF, PV_BDT, PV_BDTF, PV_D, PV_DF = 40, 41, 42, 43, 44, 45
PV_N = 46

PAD = 16

INPROJ_TILES = [(0, 128, "xs", 0), (128, 64, "xs", 1),
                (192, 128, "res", 0), (320, 64, "res", 1)]


def bcast_ap(ap, parts=128):
    """DRAM row -> all-partitions broadcast AP."""
    return bass.AP(tensor=ap.tensor, offset=ap.offset,
                   ap=[[0, parts]] + list(ap.ap))


def build_nc():
    nc = bacc.Bacc("TRN2", target_bir_lowering=False, debug=False,
                   num_devices=NCORES)

    def inp(name, shape, dt=F32):
        return nc.dram_tensor(name, shape, dt, kind="ExternalInput").ap()

    xT = inp("xT", [B, 128, KT, L])
    w_in = inp("w_in", [128, KT, 2 * DCORE])
    wxp = inp("wxp", [128, 2, 2, 80], F16)      # (p, br, ch, 80)
    wdt = inp("wdt", [48, 2, DCORE], F16)       # (p, br, d)
    pvec = inp("pvec", [128, 3, PV_N])          # (p, ch|c1fold, col)
    wout = inp("wout", [128, 3, DM], F16)       # (p, ch|c1dup, m)
    idsT = inp("idsT", [128, B, L // 128], I32)
    ident = inp("ident", [128, 128])
    idf16 = inp("idf16", [128, 128], F16)
    cdiag = inp("cdiag", [128, 2, KC, 2, 128], F16)
    wxph = inp("wxph", [128, 80], F16)          # flip ch1 xproj, rows 64:128

    outT = nc.dram_tensor("outT", [2, B, DM, L], F32, kind="ExternalOutput").ap()

    _ars = [nc.dram_tensor(f"ar_src{b}", [2, 80, L], F16).ap() for b in range(B)]
    _ard = [nc.dram_tensor(f"ar_dst{b}", [2, 80, L], F16).ap() for b in range(B)]
    ar_src = {(b, br): _ars[b][br] for b in range(B) for br in range(2)}
    ar_dst = {(b, br): _ard[b][br] for b in range(B) for br in range(2)}
    ar_full = {"src": _ars, "dst": _ard}
    xc_rows = [nc.dram_tensor(f"xc_rows{b}", [L, DCORE], F16).ap() for b in range(B)]
    bc16d = [nc.dram_tensor(f"bc16d{b}", [2, DSTATE, 2, L], F16).ap() for b in range(B)]

    with tile.TileContext(nc) as tc, contextlib.ExitStack() as ctx:
        consts = ctx.enter_context(tc.tile_pool(name="consts", bufs=1))
        persist = ctx.enter_context(tc.tile_pool(name="persist", bufs=1))
        xtp = ctx.enter_context(tc.tile_pool(name="xtp", bufs=2))
        work = ctx.enter_context(tc.tile_pool(name="work", bufs=2))
        scanp = ctx.enter_context(tc.tile_pool(name="scanp", bufs=2))
        psA = ctx.enter_context(tc.tile_pool(name="psA", bufs=2, space="PSUM"))
        psY = ctx.enter_context(tc.tile_pool(name="psY", bufs=1, space="PSUM"))

        # ---- constants ----
        s_win = consts.tile([128, KT, 2 * DCORE], F32R)
        nc.sync.dma_start(out=s_win, in_=w_in.bitcast(F32R))
        s_wxp = consts.tile([128, 2, 2, 80], F16)
        nc.sync.dma_start(out=s_wxp, in_=wxp)
        s_wdt = consts.tile([48, 2, DCORE], F16)
        nc.sync.dma_start(out=s_wdt, in_=wdt)
        s_pv = consts.tile([128, 3, PV_N], F32)
        nc.sync.dma_start(out=s_pv, in_=pvec)
        s_wout = consts.tile([128, 3, DM], F16)
        nc.sync.dma_start(out=s_wout, in_=wout)
        s_id = consts.tile([128, 128], F32)
        nc.sync.dma_start(out=s_id, in_=ident)
        s_idf = consts.tile([128, 128], F16)
        nc.sync.dma_start(out=s_idf, in_=idf16)
        s_ids = consts.tile([128, B, L // 128], I32)
        nc.sync.dma_start(out=s_ids, in_=idsT)
        s_cd = consts.tile([128, 2, KC, 2, 128], F16)
        nc.sync.dma_start(out=s_cd, in_=cdiag)
        s_wxph = consts.tile([128, 80], F16)
        nc.sync.dma_start(out=s_wxph, in_=wxph)

        xs_pad = {}
        xc = {}
        xc_c1 = {}
        res = {}
        ycomb = {}
        yflip = {}

        def ptile(name, b, shape, dt=F32):
            return persist.tile(shape, dt, name=f"{name}_{b}",
                                tag=f"{name}_{b}")

        def transpose_to_rows(src_tiles, rows_dram):
            """f16 [d-ch, L] tiles -> DRAM [L, DCORE] f16 rows."""
            for t in range(L // 128):
                for ci, (c0, cn) in enumerate(CHS):
                    pst = psA.tile([128, 128], F16, tag="ps")
                    nc.tensor.transpose(
                        pst[:, :cn], src_tiles[ci][:cn, t * 128:(t + 1) * 128],
                        s_idf[:cn, :cn])
                    srt = work.tile([128, 128], F16, name="srt", tag="srt",
                                    bufs=4)
                    nc.scalar.copy(srt[:, :cn], pst[:, :cn])
                    nc.sync.dma_start(
                        out=rows_dram[t * 128:(t + 1) * 128, c0:c0 + cn],
                        in_=srt[:, :cn])

        def gather_rows(b, rows_dram, dest_cb):
            xg = work.tile([128, L // 128, DCORE], F16, name="xg", tag="xg",
                           bufs=1)
            for t in range(L // 128):
                nc.gpsimd.indirect_dma_start(
                    out=xg[:, t, :], out_offset=None,
                    in_=rows_dram[:, :],
                    in_offset=bass.IndirectOffsetOnAxis(
                        ap=s_ids[:, b, t:t + 1], axis=0))
            for t in range(L // 128):
                for ci, (c0, cn) in enumerate(CHS):
                    pst = psA.tile([128, 128], F16, tag="ps")
                    nc.tensor.transpose(
                        pst[:cn, :], xg[:, t, c0:c0 + cn], s_idf)
                    dest_cb(ci, t, pst[:cn, :])

        def conv_silu(b, br):
            # depthwise conv as 4 shifted diag-matmuls accumulating in PSUM.
            # ch1 of both branches shares one [128, L] tile: fwd in rows
            # 0:64, flip in rows 64:128 (via tile_position col offset).
            cb0 = PV_CB if br == 0 else PV_CBF
            for ci, (c0, cn) in enumerate(CHS):
                xsp = xs_pad[(b, br, ci)]
                if ci == 0:
                    xct = ptile(f"xc{br}0", b, [128, L], F16)
                    xc[(b, br, 0)] = xct
                    pbase = 0
                else:
                    if br == 0:
                        xc_c1[b] = ptile("xcc1", b, [128, L], F16)
                    pbase = 0 if br == 0 else 64
                    xct = xc_c1[b]
                    xc[(b, br, 1)] = xc_c1[b][pbase:pbase + 64, :]
                for h in range(2):
                    pcv = psA.tile([128, 512], F32, tag="ps")
                    pslice = pcv[pbase:pbase + cn, :]
                    for j in range(KC):
                        o = PAD - (KC - 1) + j + h * 512
                        nc.tensor.matmul(
                            pslice, s_cd[:cn, br, j, ci, 0:cn],
                            xsp[:, o: o + 512],
                            start=(j == 0), stop=(j == KC - 1),
                            tile_position=(0, pbase))
                    bias = (s_pv[pbase:pbase + cn, 2, PV_CB:PV_CB + 1]
                            if ci == 1 else
                            s_pv[:cn, 0, cb0:cb0 + 1])
                    nc.scalar.activation(
                        xct[pbase:pbase + cn, h * 512:(h + 1) * 512], pslice,
                        AF.Silu, bias=bias)

        def xproj(b, br):
            for m in range(L // 128):
                pxd = psA.tile([128, 80], F32, tag="ps")
                for ci, (c0, cn) in enumerate(CHS):
                    lhsT = xc[(b, br, ci)][:, m * 128:(m + 1) * 128]
                    if ci == 1 and br == 1:
                        rhs = s_wxph[64:128, :]
                        tp = (64, 0)
                    else:
                        rhs = s_wxp[:cn, br, ci, :]
                        tp = (0, 0)
                    nc.tensor.matmul(
                        pxd, lhsT, rhs,
                        start=(ci == 0), stop=(ci == 1), tile_position=tp)
                sxd = work.tile([128, 80], F32, tag="sxd")
                nc.scalar.copy(sxd, pxd)
                pxt = psA.tile([80, 128], F32, tag="ps")
                nc.tensor.transpose(pxt, sxd[:, 0:80], s_id)
                sxt = work.tile([80, 128], F16, name="sxt", tag="sxt", bufs=2)
                nc.scalar.copy(sxt, pxt)
                nc.sync.dma_start(
                    out=ar_src[(b, br)][:, m * 128:(m + 1) * 128], in_=sxt)

        # ================= phase 1: per-batch front end =================
        for b in range(B):
            for (col0, M, kind, ci) in INPROJ_TILES:
                if kind == "xs":
                    dst = persist.tile([M, L + PAD], F16,
                                       name=f"xsp{ci}", tag=f"xsp{ci}")
                    nc.vector.memset(dst[:, 0:PAD], 0.0)
                    xs_pad[(b, 0, ci)] = dst
                else:
                    dst = ptile(f"res{ci}", b, [128, L], F16)
                    res[(b, ci)] = dst
            for h in range(2):
                xts = xtp.tile([128, KT, 512], F32R, name="xts", tag="xts")
                nc.sync.dma_start(
                    out=xts,
                    in_=xT[b, :, :, h * 512:(h + 1) * 512].bitcast(F32R))
                for (col0, M, kind, ci) in INPROJ_TILES:
                    dst = xs_pad[(b, 0, ci)] if kind == "xs" else res[(b, ci)]
                    ps = psA.tile([128, 512], F32, tag="ps")
                    psl = ps[0:M, :]
                    for k in range(KT):
                        nc.tensor.matmul(
                            psl, s_win[:, k, col0:col0 + M], xts[:, k, :],
                            start=(k == 0), stop=(k == KT - 1))
                    if kind == "xs":
                        nc.scalar.copy(
                            dst[:, PAD + h * 512:PAD + (h + 1) * 512], psl)
                    else:
                        nc.scalar.activation(
                            dst[0:M, h * 512:(h + 1) * 512], psl, AF.Silu)
            # flip rows of the duplicated ch1 res (DMA shifts partitions)
            nc.sync.dma_start(out=res[(b, 1)][64:128, :],
                              in_=res[(b, 1)][0:64, :])

            conv_silu(b, 0)
            xproj(b, 0)

            transpose_to_rows([xc[(b, 0, 0)], xc[(b, 0, 1)]], xc_rows[b])
            for ci, (c0, cn) in enumerate(CHS):
                dst = persist.tile([cn, L + PAD], F16,
                                   name=f"xspf{ci}", tag=f"xspf{ci}")
                nc.vector.memset(dst[:, 0:PAD], 0.0)
                xs_pad[(b, 1, ci)] = dst

            def xg_dest(ci, t, ps_ap, b=b):
                nc.scalar.copy(
                    xs_pad[(b, 1, ci)][:, PAD + t * 128:PAD + (t + 1) * 128],
                    ps_ap)
            gather_rows(b, xc_rows[b], xg_dest)
            conv_silu(b, 1)
            xproj(b, 1)
            nc.gpsimd.collective_compute(
                "AllReduce", OP.add,
                replica_groups=[list(range(NCORES))],
                ins=[ar_full["src"][b]], outs=[ar_full["dst"][b]])

        # ========== phase 2: ssm per batch, both branches fused ==========
        # ch0 of each branch runs as its own [128, L] pipeline; ch1 of BOTH
        # branches shares [128, L] tiles (fwd rows 0:64, flip rows 64:128),
        # so the scan/exp/t1 run once for the pair.
        y_c1 = {}

        def phase2(b):
            sxdT = {}
            for br in range(2):
                sx = scanp.tile([48, L], F16, name="sxdT", tag=f"sxdT{br}", bufs=1)
                nc.sync.dma_start(out=sx, in_=ar_dst[(b, br)][0:48, :])
                sxdT[br] = sx
                sbc16 = scanp.tile([16, 2, L], F16, name="sbc16",
                                   tag=f"sbc16{br}", bufs=1)
                bc_src = ar_dst[(b, br)][48:80, :].rearrange(
                    "(j n) l -> n j l", j=2)
                nc.sync.dma_start(out=sbc16, in_=bc_src)
                nc.sync.dma_start(out=bc16d[b][br], in_=sbc16)

            # dt_proj -> softplus(exp+ln) -> delta (f16)
            delta, du = {}, {}
            ets = {}
            for br in range(2):
                dl = scanp.tile([128, L], F16, name="delta", tag=f"delta0{br}", bufs=1)
                bcol = PV_BDT if br == 0 else PV_BDTF
                for h in range(2):
                    pdt = psA.tile([128, 512], F32, tag="ps")
                    nc.tensor.matmul(
                        pdt, s_wdt[0:48, br, 0:128],
                        sxdT[br][0:48, h * 512:(h + 1) * 512],
                        start=True, stop=True)
                    et = work.tile([128, 512], F32, name="et",
                                   tag=f"sp{br}{h}", bufs=1)
                    nc.scalar.activation(et, pdt, AF.Exp,
                                         bias=s_pv[:, 0, bcol:bcol + 1])
                    ets[(br, h)] = et
                delta[f"0{br}"] = dl
            dlc = scanp.tile([128, L], F16, name="delta", tag="deltac1", bufs=1)
            for h in range(2):
                pdt = psA.tile([128, 512], F32, tag="ps")
                nc.tensor.matmul(
                    pdt[0:64, :], s_wdt[0:48, 0, 128:192],
                    sxdT[0][0:48, h * 512:(h + 1) * 512],
                    start=True, stop=True, tile_position=(0, 0))
                nc.tensor.matmul(
                    pdt[64:128, :], s_wdt[0:48, 1, 128:192],
                    sxdT[1][0:48, h * 512:(h + 1) * 512],
                    start=True, stop=True, tile_position=(0, 64))
                et = work.tile([128, 512], F32, name="et", tag=f"spc{h}",
                               bufs=1)
                nc.scalar.activation(et, pdt, AF.Exp,
                                     bias=s_pv[:, 2, PV_BDT:PV_BDT + 1])
                ets[("c", h)] = et
            delta["c1"] = dlc
            for br in range(2):
                for h in range(2):
                    nc.scalar.activation(
                        delta[f"0{br}"][:, h * 512:(h + 1) * 512],
                        ets[(br, h)], AF.Ln, bias=1.0)
            for h in range(2):
                nc.scalar.activation(
                    dlc[:, h * 512:(h + 1) * 512], ets[("c", h)],
                    AF.Ln, bias=1.0)

            for br in range(2):
                dut = scanp.tile([128, L], F16, name="du", tag=f"du0{br}", bufs=1)
                nc.vector.tensor_mul(dut, delta[f"0{br}"], xc[(b, br, 0)])
                du[f"0{br}"] = dut
            duc = scanp.tile([128, L], F16, name="du", tag="duc1", bufs=1)
            nc.vector.tensor_mul(duc, dlc, xc_c1[b])
            du["c1"] = duc

            py = {k: psY.tile([128, L], F32, name="py", tag=f"py{k}")
                  for k in ("00", "01", "c1")}
            for n in range(DSTATE):
                sBC = {}
                for br in range(2):
                    sB = work.tile([128, L], F16, name="sB", tag=f"sB{br}",
                                   bufs=3)
                    nc.sync.dma_start(out=sB,
                                      in_=bcast_ap(bc16d[b][br, n, 0]))
                    sC = work.tile([128, L], F16, name="sC", tag=f"sC{br}",
                                   bufs=3)
                    nc.sync.dma_start(out=sC,
                                      in_=bcast_ap(bc16d[b][br, n, 1]))
                    sBC[br] = (sB, sC)

                sBm = work.tile([128, L], F16, name="sBm", tag="sBm", bufs=2)
                nc.sync.dma_start(out=sBm[0:64, :], in_=sBC[0][0][0:64, :])
                nc.sync.dma_start(out=sBm[64:128, :], in_=sBC[1][0][64:128, :])
                sCm = work.tile([128, L], F16, name="sCm", tag="sCm", bufs=2)
                nc.sync.dma_start(out=sCm[0:64, :], in_=sBC[0][1][0:64, :])
                nc.sync.dma_start(out=sCm[64:128, :], in_=sBC[1][1][64:128, :])

                hx = {}
                for br in range(2):
                    acol = (PV_A if br == 0 else PV_AF) + n
                    dbu = scanp.tile([128, L], F16, name="dbu",
                                     tag=f"dbu0{br}", bufs=1)
                    nc.vector.tensor_mul(dbu, du[f"0{br}"], sBC[br][0])
                    dA = scanp.tile([128, L], F16, name="dA", tag=f"dA0{br}", bufs=2)
                    nc.scalar.activation(dA, delta[f"0{br}"], AF.Exp,
                                         scale=s_pv[:, 0, acol:acol + 1])
                    h_t = scanp.tile([128, L], F16, name="h", tag=f"h0{br}", bufs=1)
                    nc.vector.tensor_tensor_scan(
                        h_t, dA, dbu, 0.0, op0=OP.mult, op1=OP.add)
                    hc = scanp.tile([128, L], F16, name="hc", tag=f"hc0{br}", bufs=2)
                    nc.vector.tensor_mul(hc, h_t, sBC[br][1])
                    hx[f"0{br}"] = hc
                dbuc = scanp.tile([128, L], F16, name="dbu", tag="dbuc1", bufs=1)
                nc.vector.tensor_mul(dbuc, duc, sBm)
                dAc = scanp.tile([128, L], F16, name="dA", tag="dAc1", bufs=2)
                acol = PV_A + n
                nc.scalar.activation(dAc, dlc, AF.Exp,
                                     scale=s_pv[:, 2, acol:acol + 1])
                hct = scanp.tile([128, L], F16, name="h", tag="hc1", bufs=1)
                nc.vector.tensor_tensor_scan(
                    hct, dAc, dbuc, 0.0, op0=OP.mult, op1=OP.add)
                hcc = scanp.tile([128, L], F16, name="hc", tag="hcc1", bufs=2)
                nc.vector.tensor_mul(hcc, hct, sCm)
                hx["c1"] = hcc

                for k in ("00", "01", "c1"):
                    for h2 in range(2):
                        hs = slice(h2 * 512, (h2 + 1) * 512)
                        nc.tensor.matmul(
                            py[k][:, hs], s_idf, hx[k][:, hs],
                            start=(n == 0), stop=(n == DSTATE - 1))

            # y = (py + u*D) * res
            for br in range(2):
                dcol = PV_D if br == 0 else PV_DF
                t1 = scanp.tile([128, L], F16, name="t1", tag=f"dA0{br}", bufs=2)
                nc.vector.scalar_tensor_tensor(
                    t1, xc[(b, br, 0)], s_pv[:, 0, dcol:dcol + 1],
                    py[f"0{br}"], op0=OP.mult, op1=OP.add)
                dstd = ycomb if br == 0 else yflip
                yt = ptile("ycomb0" if br == 0 else "yflip0", b, [128, L], F16)
                nc.vector.tensor_mul(yt, t1, res[(b, 0)])
                dstd[(b, 0)] = yt
            t1c = scanp.tile([128, L], F16, name="t1", tag="dAc1", bufs=2)
            nc.vector.scalar_tensor_tensor(
                t1c, xc_c1[b], s_pv[:, 2, PV_D:PV_D + 1],
                py["c1"], op0=OP.mult, op1=OP.add)
            yc1 = ptile("yc1", b, [128, L], F16)
            nc.vector.tensor_mul(yc1, t1c, res[(b, 1)])
            y_c1[b] = yc1

        # ========== phase 3: out_proj (f16), fwd + flip partials ==========
        def out_proj(b):
            for wi in range(2):
                y0 = (ycomb if wi == 0 else yflip)[(b, 0)]
                c1b, c1w, c1tp = ((0, 1, (0, 0)) if wi == 0
                                  else (64, 2, (64, 0)))
                for m in range(DM // 128):
                    for h in range(2):
                        po = psA.tile([128, 512], F32, tag="ps")
                        nc.tensor.matmul(
                            po, s_wout[:128, 0, m * 128:(m + 1) * 128],
                            y0[:, h * 512:(h + 1) * 512],
                            start=True, stop=False)
                        nc.tensor.matmul(
                            po, s_wout[c1b:c1b + 64, c1w,
                                       m * 128:(m + 1) * 128],
                            y_c1[b][c1b:c1b + 64, h * 512:(h + 1) * 512],
                            start=False, stop=True, tile_position=c1tp)
                        so = work.tile([128, 512], F32, name="so", tag="so",
                                       bufs=2)
                        nc.scalar.copy(so, po)
                        nc.sync.dma_start(
                            out=outT[wi, b, m * 128:(m + 1) * 128,
                                     h * 512:(h + 1) * 512],
                            in_=so)

        for b in range(B):
            phase2(b)
        for b in range(B):
            out_proj(b)

    nc.compile()
    return nc


_NC_CACHE = None


def _get_nc():
    global _NC_CACHE
    if _NC_CACHE is None:
        _NC_CACHE = build_nc()
    return _NC_CACHE


def _chunk2(v):
    out = np.zeros((128, 2) + v.shape[1:], v.dtype)
    out[:, 0] = v[0:128]
    out[:64, 1] = v[128:192]
    return out


def _prep_inputs(inputs):
    g = {k: np.asarray(v) for k, v in inputs.items()}
    x = g["x"].astype(np.float32, copy=False)
    ids = g["x_flip_ids"].astype(np.int32)
    A = -np.exp(g["A_log"].astype(np.float32))
    A_f = -np.exp(g["A_log_f"].astype(np.float32))

    xT = np.ascontiguousarray(
        x.transpose(0, 2, 1).reshape(B, KT, 128, L).transpose(0, 2, 1, 3))
    idsT = np.ascontiguousarray(
        ids.reshape(B, L // 128, 128).transpose(2, 0, 1))
    ident = np.eye(128, dtype=np.float32)
    idf16 = np.eye(128, dtype=np.float16)

    in_maps = []
    for c in range(NCORES):
        sl = slice(c * DCORE, (c + 1) * DCORE)
        W_in = g["W_in"]
        xs_c = W_in[:, sl]
        rs_c = W_in[:, DI + c * DCORE: DI + (c + 1) * DCORE]
        w384 = np.concatenate([xs_c, rs_c], axis=1).astype(np.float32)
        w_in_t = np.ascontiguousarray(
            w384.reshape(KT, 128, 2 * DCORE).transpose(1, 0, 2))

        wxp_c = np.ascontiguousarray(np.stack(
            [_chunk2(g["W_xproj"][sl].astype(np.float16)),
             _chunk2(g["W_xproj_f"][sl].astype(np.float16))], axis=1))
        wdt_c = np.ascontiguousarray(np.stack(
            [g["W_dt"][:, sl].astype(np.float16),
             g["W_dt_f"][:, sl].astype(np.float16)], axis=1))
        w_out16 = g["W_out"][sl].astype(np.float16)
        wout_c = np.zeros((128, 3, DM), np.float16)
        wout_c[:, 0:2] = _chunk2(w_out16)
        wout_c[64:128, 2] = w_out16[128:192]
        wout_c = np.ascontiguousarray(wout_c)
        wxph_c = np.zeros((128, 80), np.float16)
        wxph_c[64:128] = g["W_xproj_f"][sl].astype(np.float16)[128:192]

        cd = np.zeros((128, 2, KC, 2, 128), np.float16)
        for bri, cwk in enumerate(["conv_w", "conv_w_f"]):
            w = g[cwk][sl, 0, :]  # (192, 4)
            for j in range(KC):
                cd[:, bri, j, 0, :][np.diag_indices(128)] = w[0:128, j]
                cd[:64, bri, j, 1, :64][np.diag_indices(64)] = w[128:192, j]
        pv = np.zeros((DCORE, PV_N), np.float32)
        pv[:, PV_CW:PV_CW + KC] = g["conv_w"][sl, 0, :]
        pv[:, PV_CWF:PV_CWF + KC] = g["conv_w_f"][sl, 0, :]
        pv[:, PV_A:PV_A + DSTATE] = A[sl]
        pv[:, PV_AF:PV_AF + DSTATE] = A_f[sl]
        pv[:, PV_CB] = g["conv_b"][sl]
        pv[:, PV_CBF] = g["conv_b_f"][sl]
        pv[:, PV_BDT] = g["b_dt"][sl]
        pv[:, PV_BDTF] = g["b_dt_f"][sl]
        pv[:, PV_D] = g["D"][sl]
        pv[:, PV_DF] = g["D_f"][sl]
        pv3 = np.zeros((128, 3, PV_N), np.float32)
        pv3[:, 0:2] = _chunk2(pv)
        hi = slice(c * DCORE + 128, (c + 1) * DCORE)
        pv3[0:64, 2, PV_CB] = g["conv_b"][hi]
        pv3[64:128, 2, PV_CB] = g["conv_b_f"][hi]
        pv3[0:64, 2, PV_BDT] = g["b_dt"][hi]
        pv3[64:128, 2, PV_BDT] = g["b_dt_f"][hi]
        pv3[0:64, 2, PV_A:PV_A + DSTATE] = A[hi]
        pv3[64:128, 2, PV_A:PV_A + DSTATE] = A_f[hi]
        pv3[0:64, 2, PV_D] = g["D"][hi]
        pv3[64:128, 2, PV_D] = g["D_f"][hi]
        pvec_c = np.ascontiguousarray(pv3)

        in_maps.append(dict(
            xT=xT, w_in=w_in_t, wxp=wxp_c, wdt=wdt_c, pvec=pvec_c,
            wout=wout_c, idsT=idsT, ident=ident, idf16=idf16,
            cdiag=cd, wxph=wxph_c))
    return in_maps


def kernel(**inputs):
    nc = _get_nc()
    in_maps = _prep_inputs(inputs)
    ids = np.asarray(inputs["x_flip_ids"]).astype(np.int64)
    res = run_bass_kernel_spmd(nc, in_maps, core_ids=list(range(NCORES)))
    acc = np.zeros((2, B, DM, L), np.float64)
    for r in res.results:
        acc += r["outT"].astype(np.float64)
    out = acc[0]
    for b in range(B):
        out[b] += acc[1, b][:, ids[b]]
    return np.ascontiguousarray(out.transpose(0, 2, 1)).astype(np.float32)

